# revision 5
# baseline (speedup 1.0000x reference)
"""CBOW negative-sampling loss on 8 Trainium2 NeuronCores — v5.

v3 (189us): DVE did the elementwise row*tgt multiply (~160us busy) and
all 38MB bf16 streamed on one HWDGE ring (114us).

v5 strategy:
  - fp8(e4m3) packing of gathered rows + targets: 19MB/core, DMA floor
    ~53us at 358GB/s, split across both HWDGE rings (qSP: main rows,
    qACT: tail rows + stationaries).
  - TensorE computes the dot products DIRECTLY via DoubleRow fp8
    matmuls (0.5 cyc per output column): per 128-sample tile, 4 groups
    of 32 samples. Group q's stationary is a FULL-WIDTH [*, 2, 128]
    tensor with the 32 target vectors in columns 32q:32q+32 and zeros
    elsewhere (DoubleRow only allows PSUM dst partition 0, so groups
    stack on partitions by accumulating into the same [128, 480]
    region; the zero columns contribute nothing). Two passes per
    group/half: d 0:256 as [128,2,480] and d 256:300 as [22,2,480].
  - Off-diagonal kill INSIDE the matmul: a constant fp8 "mask matmul"
    ([17,2] contraction: 32 identity rows + 1 const row, values +-240)
    opens each accumulation group, adding exactly 0 to diagonal cells
    (m%32 == 16h+b) and -57600 elsewhere. PSUM ends up holding the
    logit on diagonal cells and <= -57000 elsewhere.
  - Extraction = ONE DVE tensor_reduce(max) per tile straight from
    PSUM over the (b, h) axes -> L[:, t, 30] bf16.
  - Post-pass: sign flip, stable softplus (Exp/Ln on ACT), weighted
    accumulate -> [128,1]; host sums /B.
"""

import sys

for _p in ("/opt/trn_rl_repo", "/opt/pypackages"):
    if _p not in sys.path:
        sys.path.append(_p)

import ml_dtypes
import numpy as np

import concourse.bass as bass
import concourse.bacc as bacc
import concourse.tile as tile
from concourse import mybir
from concourse.bass_utils import run_bass_kernel_spmd

V = 100000
D = 300
B = 16384
C = 10
K = 20
NCORES = 8
P = 128
NJ = C + K  # 30
BCORE = B // NCORES  # 2048
NT = BCORE // P  # 16
D2 = 22  # d 256:300 as [22, 2]
W1 = 2 * NJ * P  # 7680
MG = 240.0  # fp8-exact mask magnitude; mask adds -MG*MG off-diagonal

F8NP = ml_dtypes.float8_e4m3
F8 = mybir.dt.float8e4
BF = mybir.dt.bfloat16
_f32 = mybir.dt.float32
DR = mybir.MatmulPerfMode.DoubleRow


def _ap(sliced, dims):
    return bass.AP(sliced.tensor, sliced.offset, [sliced.ap[0], *dims])


def build_nc(nt: int):
    nc = bacc.Bacc(None, target_bir_lowering=False, debug=False)
    AF = mybir.ActivationFunctionType
    OP = mybir.AluOpType

    mv01 = nc.dram_tensor("mv01", [nt * P, W1], F8, kind="ExternalInput")
    mv2 = nc.dram_tensor("mv2", [nt * D2, W1], F8, kind="ExternalInput")
    st01d = nc.dram_tensor("st01", [nt * P, 256], F8, kind="ExternalInput")
    st2d = nc.dram_tensor("st2", [nt * D2, 256], F8, kind="ExternalInput")
    mAd = nc.dram_tensor("maskA", [17, 256], F8, kind="ExternalInput")
    mBd = nc.dram_tensor("maskB", [17, 2 * 960], F8, kind="ExternalInput")
    sgnd = nc.dram_tensor("sgn", [P, NJ], BF, kind="ExternalInput")
    wzd = nc.dram_tensor("wz", [P, NJ], _f32, kind="ExternalInput")
    out = nc.dram_tensor("out", [P, 1], _f32, kind="ExternalOutput")

    with tile.TileContext(nc) as tc:
        with (
            tc.tile_pool(name="g1p", bufs=4) as g1p,
            tc.tile_pool(name="g2p", bufs=4) as g2p,
            tc.tile_pool(name="pp", bufs=2, space="PSUM") as pp,
            tc.tile_pool(name="singles", bufs=1) as singles,
        ):
            mA = singles.tile([17, 2, P], F8)
            nc.scalar.dma_start(out=mA[:], in_=mAd[:])
            mB = singles.tile([17, 2, 960], F8)
            nc.scalar.dma_start(out=mB[:], in_=mBd[:])
            sgn = singles.tile([P, NJ], BF)
            nc.scalar.dma_start(out=sgn[:], in_=sgnd[:])
            wz = singles.tile([P, NJ], _f32)
            nc.scalar.dma_start(out=wz[:], in_=wzd[:])

            # manual A/B double-buffered zero-padded stationaries:
            # [d, k, q, 128] with cols 32q:32q+32 of plane q holding the
            # group's 32 target vectors, zeros elsewhere (memset once).
            stb = []
            st2b = []
            for i in range(2):
                s_ = singles.tile([P, 2, 4, P], F8, tag=f"stb{i}")
                nc.vector.memset(s_[:], 0.0)
                stb.append(s_)
                s2_ = singles.tile([D2, 2, 4, P], F8, tag=f"st2b{i}")
                nc.vector.memset(s2_[:], 0.0)
                st2b.append(s2_)

            L = singles.tile([P, nt, NJ], BF)

            for t in range(nt):
                g1 = g1p.tile([P, 2, NJ * P], F8, tag="g1")
                nc.sync.dma_start(out=g1[:], in_=mv01[t * P : (t + 1) * P])
                g2 = g2p.tile([D2, 2, NJ * P], F8, tag="g2")
                nc.scalar.dma_start(out=g2[:], in_=mv2[t * D2 : (t + 1) * D2])

                st = stb[t % 2]
                st2 = st2b[t % 2]
                for k in range(2):
                    # dram rows (t, d), cols k*128 + (q*32 + m); group
                    # q's block lands at plane-q col offset 32q, so the
                    # q-stride in the flat (q, 128) space is 128+32=160
                    nc.sync.dma_start(
                        out=_ap(st[:, k, 0, 0:32], [[160, 4], [1, 32]]),
                        in_=st01d[t * P : (t + 1) * P, k * P : (k + 1) * P],
                    )
                    nc.scalar.dma_start(
                        out=_ap(st2[:, k, 0, 0:32], [[160, 4], [1, 32]]),
                        in_=st2d[t * D2 : (t + 1) * D2, k * P : (k + 1) * P],
                    )

                ps = pp.tile([P, 2, 512], _f32, tag="ps")
                for h in range(2):
                    o = ps[:, h, 0 : 16 * NJ]
                    nc.tensor.matmul(
                        o,
                        mA[:],
                        mB[:, :, 480 * h : 480 * (h + 1)],
                        start=True,
                        stop=False,
                        perf_mode=DR,
                    )
                    for q in range(4):
                        c0 = 960 * q + 480 * h
                        nc.tensor.matmul(
                            o,
                            st[:, :, q, :],
                            g1[:, :, c0 : c0 + 480],
                            start=False,
                            stop=False,
                            perf_mode=DR,
                        )
                        nc.tensor.matmul(
                            o,
                            st2[:, :, q, :],
                            g2[:, :, c0 : c0 + 480],
                            start=False,
                            stop=(q == 3),
                            perf_mode=DR,
                        )

                # psum cell [m, h*512 + b*30 + j] = logit if m%32==16h+b
                # else <= -57000; reduce max over (b, h) -> logits
                nc.vector.tensor_reduce(
                    out=L[:, t, :],
                    in_=_ap(ps[:], [[1, NJ], [NJ, 16], [512, 2]]),
                    axis=mybir.AxisListType.XY,
                    op=OP.max,
                )

            # ---- post-pass: [P, nt*30] ----
            def bc(a, n):
                return bass.AP(a.tensor, a.offset, [a.ap[0], [0, n], a.ap[-1]])

            z = singles.tile([P, nt, NJ], BF)
            nc.vector.tensor_tensor(
                out=z[:], in0=L[:], in1=bc(sgn[:], nt), op=OP.mult
            )
            rl = singles.tile([P, nt, NJ], BF)
            nc.vector.tensor_scalar_max(rl[:], z[:], 0.0)
            na = singles.tile([P, nt, NJ], BF)
            nc.vector.scalar_tensor_tensor(
                out=na[:],
                in0=z[:],
                scalar=-1.0,
                in1=z[:],
                op0=OP.mult,
                op1=OP.min,
            )
            e = singles.tile([P, nt, NJ], _f32)
            nc.scalar.activation(e[:], na[:], AF.Exp)
            l = singles.tile([P, nt, NJ], _f32)
            nc.scalar.activation(l[:], e[:], AF.Ln, bias=1.0)
            sp = singles.tile([P, nt, NJ], _f32)
            nc.vector.tensor_tensor(out=sp[:], in0=rl[:], in1=l[:], op=OP.add)
            spw = singles.tile([P, nt, NJ], _f32)
            acc = singles.tile([P, 1], _f32)
            nc.vector.scalar_tensor_tensor(
                out=spw[:],
                in0=sp[:],
                scalar=1.0,
                in1=bc(wz[:], nt),
                op0=OP.mult,
                op1=OP.mult,
                accum_out=acc[:],
            )
            nc.sync.dma_start(out=out[:], in_=acc[:])

    nc.compile()
    return nc


_NC_CACHE: dict = {}


def _get_nc(nt: int):
    if nt not in _NC_CACHE:
        _NC_CACHE[nt] = build_nc(nt)
    return _NC_CACHE[nt]


def kernel(i_emb, o_emb, context, target, neg_samples, _trace=False, _trace_kwargs=None):
    i_emb = np.asarray(i_emb, dtype=np.float32)
    o_emb = np.asarray(o_emb, dtype=np.float32)
    context = np.asarray(context).astype(np.int64)
    target = np.asarray(target).astype(np.int64)
    neg_samples = np.asarray(neg_samples).astype(np.int64)

    o8 = o_emb.astype(F8NP)
    i8 = i_emb.astype(F8NP)

    allj = np.concatenate([context, neg_samples], axis=1)  # [B, 30]
    rows = o8[allj]  # [B, 30, 300]
    tg = i8[target]  # [B, 300]

    r5 = rows.reshape(NCORES, NT, P, NJ, D)
    # mv01[c, t, d, k, s, j] = r5[c, t, s, j, 128k + d]
    m1 = r5[..., : 2 * P].reshape(NCORES, NT, P, NJ, 2, P)
    mv01 = np.ascontiguousarray(m1.transpose(0, 1, 5, 4, 2, 3)).reshape(
        NCORES, NT * P, W1
    )
    # mv2[c, t, d2, k2, s, j] = r5[c, t, s, j, 256 + 22*k2 + d2]
    m2 = r5[..., 2 * P : D].reshape(NCORES, NT, P, NJ, 2, D2)
    mv2 = np.ascontiguousarray(m2.transpose(0, 1, 5, 4, 2, 3)).reshape(
        NCORES, NT * D2, W1
    )

    t4 = tg.reshape(NCORES, NT, P, D)
    # st01[c, (t, d), (k, s)] = tg[c, t, s, 128k + d]
    s1 = t4[..., : 2 * P].reshape(NCORES, NT, P, 2, P)
    st01 = np.ascontiguousarray(s1.transpose(0, 1, 4, 3, 2)).reshape(
        NCORES, NT * P, 256
    )
    s2 = t4[..., 2 * P : D].reshape(NCORES, NT, P, 2, D2)
    st2 = np.ascontiguousarray(s2.transpose(0, 1, 4, 3, 2)).reshape(
        NCORES, NT * D2, 256
    )

    # mask matmul constants: rows v = (s, k); A[s, k, m]:
    #   s<16: 240 * delta(m%32 == 16k + s)
    #   s=16, k=0: -240 for all m; k=1: 0
    # B[s, k, (h, b, j)]: s<16: 240 * delta(k==h) * delta(b==s);
    #   s=16, k=0: 240; k=1: 0
    mm = np.arange(P)
    mA = np.zeros((17, 2, P), np.float32)
    for s_ in range(16):
        for k_ in range(2):
            mA[s_, k_] = MG * ((mm % 32) == (16 * k_ + s_))
    mA[16, 0] = -MG
    mB = np.zeros((17, 2, 2, 16, NJ), np.float32)
    for s_ in range(16):
        for h_ in range(2):
            mB[s_, h_, h_, s_, :] = MG
    mB[16, 0] = MG

    jj = np.arange(NJ)
    sgn_row = np.where(jj < C, -1.0, 1.0).astype(ml_dtypes.bfloat16)
    wz_row = np.where(jj < C, 1.0 / C, 1.0).astype(np.float32)
    consts = {
        "maskA": np.ascontiguousarray(mA.astype(F8NP)).reshape(17, 256),
        "maskB": np.ascontiguousarray(mB.astype(F8NP)).reshape(17, 2 * 960),
        "sgn": np.tile(sgn_row, (P, 1)),
        "wz": np.tile(wz_row, (P, 1)),
    }

    nc = _get_nc(NT)

    in_maps = []
    for c in range(NCORES):
        in_maps.append(
            {
                "mv01": mv01[c],
                "mv2": mv2[c],
                "st01": st01[c],
                "st2": st2[c],
                **consts,
            }
        )

    kw = {}
    if _trace:
        kw["trace"] = True
        if _trace_kwargs:
            kw.update(_trace_kwargs)
    res = run_bass_kernel_spmd(nc, in_maps, core_ids=list(range(NCORES)), **kw)

    total = np.float64(0.0)
    for c in range(NCORES):
        total += np.asarray(res.results[c]["out"], dtype=np.float64).sum()
    loss = np.float32(total / B)
    if _trace:
        return loss, res
    return loss


# revision 7
# speedup vs baseline: 2.0062x; 2.0062x over previous
"""CBOW negative-sampling loss on 8 Trainium2 NeuronCores — v6.

Measured constraints driving this design (HW microbenchmarks):
  - per-core DMA ceiling ~330 GB/s (two HWDGE rings help <15%): fp8
    packing is mandatory (19MB/core -> ~58us floor; bf16 would be 117).
  - DVE is 2 elem/lane/cyc only for 2-byte dtypes (fp8 runs 1x).
  - DoubleRow fp8 matmul: 480-col MM sustains ~365ns back-to-back
    (200ns stream + ~165ns fixed); LDWEIGHTS hides behind the previous
    MM. DR requires PSUM dst partition offset 0.

Pipeline per 128-sample tile (4 groups of 32 samples):
  - PE (8 DR matmuls): d 0:256 dot-product parts. Group q's stationary
    is full-width [128,2,128] with the 32 target vectors at columns
    32q:32q+32, zeros elsewhere (memset-once A/B buffers; per-tile DMA
    rewrites only the data blocks). Groups accumulate into one
    [128, 2x480] PSUM region; cell [m, h, b*30+j] holds
    tgt_m . row_{32(m//32)+16h+b}[0:256] — diagonal m%32==16h+b is the
    wanted partial logit.
  - ACT: evacuate PSUM -> bf16 Y reordered to [j, (h,b)] layout (free
    strided reorder), and upcast the fp8 d 256:300 tail rows to bf16.
  - DVE (all 2x mode): Y + maskconst (0 on diagonal, -1e30 off) then
    tensor_reduce(max) over (h,b) -> L01[:, t, 30]. Tail: bf16
    multiply by broadcast target tail + reduce(add) over d44 ->
    Ltail[:, t, 30].
  - Post: L = L01 + Ltail, sign flip, stable softplus, weighted
    accumulate -> [128,1]; host sums across cores /B.
"""

import sys

for _p in ("/opt/trn_rl_repo", "/opt/pypackages"):
    if _p not in sys.path:
        sys.path.append(_p)

import ml_dtypes
import numpy as np

import concourse.bass as bass
import concourse.bacc as bacc
import concourse.tile as tile
from concourse import mybir
from concourse.bass_utils import run_bass_kernel_spmd

V = 100000
D = 300
B = 16384
C = 10
K = 20
NCORES = 8
P = 128
NJ = C + K  # 30
BCORE = B // NCORES  # 2048
NT = BCORE // P  # 16
DT = 44  # tail dims 256:300
W1 = 2 * NJ * P  # 7680
WT = NJ * DT  # 1320
NEG_INF = -1.0e30

F8NP = ml_dtypes.float8_e4m3
BFNP = ml_dtypes.bfloat16
F8 = mybir.dt.float8e4
BF = mybir.dt.bfloat16
_f32 = mybir.dt.float32
DR = mybir.MatmulPerfMode.DoubleRow


def _ap(sliced, dims):
    return bass.AP(sliced.tensor, sliced.offset, [sliced.ap[0], *dims])


def build_nc(nt: int):
    nc = bacc.Bacc(None, target_bir_lowering=False, debug=False)
    AF = mybir.ActivationFunctionType
    OP = mybir.AluOpType

    mv01 = nc.dram_tensor("mv01", [nt * P, W1], F8, kind="ExternalInput")
    taild = nc.dram_tensor("tail", [nt * P, WT], F8, kind="ExternalInput")
    st01d = nc.dram_tensor("st01", [nt * P, 256], F8, kind="ExternalInput")
    ttd = nc.dram_tensor("tt", [P, nt * DT], F8, kind="ExternalInput")
    maskd = nc.dram_tensor("maskadd", [P, 16 * NJ], BF, kind="ExternalInput")
    sgnd = nc.dram_tensor("sgn", [P, NJ], BF, kind="ExternalInput")
    wzd = nc.dram_tensor("wz", [P, NJ], _f32, kind="ExternalInput")
    out = nc.dram_tensor("out", [P, 1], _f32, kind="ExternalOutput")

    with tile.TileContext(nc) as tc:
        with (
            tc.tile_pool(name="g1p", bufs=4) as g1p,
            tc.tile_pool(name="tlp", bufs=4) as tlp,
            tc.tile_pool(name="tbp", bufs=2) as tbp,
            tc.tile_pool(name="tpp", bufs=2) as tpp,
            tc.tile_pool(name="yp", bufs=2) as yp,
            tc.tile_pool(name="mp", bufs=2) as mp,
            tc.tile_pool(name="pp", bufs=2, space="PSUM") as pp,
            tc.tile_pool(name="singles", bufs=1) as singles,
        ):
            mask = singles.tile([P, 16 * NJ], BF)
            nc.scalar.dma_start(out=mask[:], in_=maskd[:])
            sgn = singles.tile([P, NJ], BF)
            nc.scalar.dma_start(out=sgn[:], in_=sgnd[:])
            wz = singles.tile([P, NJ], _f32)
            nc.scalar.dma_start(out=wz[:], in_=wzd[:])
            tt8 = singles.tile([P, nt, DT], F8)
            nc.scalar.dma_start(out=tt8[:], in_=ttd[:])
            ttb = singles.tile([P, nt, DT], BF)
            nc.scalar.activation(ttb[:], tt8[:], AF.Copy)

            # A/B zero-padded stationaries: [d, k, q, 128], data blocks
            # at plane-q cols 32q:32q+32 (flat (q,128)-space stride 160)
            stb = []
            for i in range(2):
                s_ = singles.tile([P, 2, 8, P], F8, tag=f"stb{i}")
                nc.vector.memset(s_[:], 0.0)
                stb.append(s_)

            L01 = singles.tile([P, nt, NJ], BF)
            Ltl = singles.tile([P, nt, NJ], BF)

            for t in range(nt):
                g1 = g1p.tile([P, 2, NJ * P], F8, tag="g1")
                nc.sync.dma_start(
                    out=g1[:, 0, :], in_=mv01[t * P : (t + 1) * P, 0 : NJ * P]
                )
                nc.scalar.dma_start(
                    out=g1[:, 1, :],
                    in_=mv01[t * P : (t + 1) * P, NJ * P : W1],
                )
                tl8 = tlp.tile([P, NJ, DT], F8, tag="tl8")
                nc.sync.dma_start(out=tl8[:], in_=taild[t * P : (t + 1) * P])

                st = stb[t % 2]
                for k in range(2):
                    nc.sync.dma_start(
                        out=_ap(st[:, k, 0, 0:16], [[144, 8], [1, 16]]),
                        in_=st01d[t * P : (t + 1) * P, k * P : (k + 1) * P],
                    )

                ps = pp.tile([P, 512], _f32, tag="ps")
                o = ps[:, 0 : 16 * NJ]
                for q in range(8):
                    c0 = 480 * q
                    nc.tensor.matmul(
                        o,
                        st[:, :, q, :],
                        g1[:, :, c0 : c0 + 480],
                        start=(q == 0),
                        stop=(q == 7),
                        perf_mode=DR,
                    )

                # ACT: contiguous evac psum -> Y bf16 [P, (b, j)]
                Y = yp.tile([P, 16 * NJ], BF, tag="Y")
                nc.scalar.activation(Y[:], ps[:, 0 : 16 * NJ], AF.Copy)
                # ACT: upcast tail rows fp8 -> bf16
                tlb = tbp.tile([P, NJ, DT], BF, tag="tlb")
                nc.scalar.activation(tlb[:], tl8[:], AF.Copy)

                # DVE: masked max-extraction of the diagonal (b == p%16)
                M = mp.tile([P, 16 * NJ], BF, tag="M")
                nc.vector.tensor_tensor(
                    out=M[:], in0=Y[:], in1=mask[:], op=OP.add
                )
                nc.vector.tensor_reduce(
                    out=L01[:, t, :],
                    in_=_ap(M[:], [[1, NJ], [NJ, 16]]),
                    axis=mybir.AxisListType.X,
                    op=OP.max,
                )

                # GPSIMD: tail products; DVE: reduce over d44
                tp = tpp.tile([P, NJ, DT], BF, tag="tp")
                nc.gpsimd.tensor_tensor(
                    out=tp[:],
                    in0=tlb[:],
                    in1=_ap(ttb[:, t, :], [[0, NJ], [1, DT]]),
                    op=OP.mult,
                )
                with nc.allow_low_precision("44-term tail sums fit bf16"):
                    nc.vector.tensor_reduce(
                        out=Ltl[:, t, :],
                        in_=tp[:],
                        axis=mybir.AxisListType.X,
                        op=OP.add,
                    )

            # ---- post-pass: [P, nt*30] ----
            def bc(a, n):
                return bass.AP(a.tensor, a.offset, [a.ap[0], [0, n], a.ap[-1]])

            L = singles.tile([P, nt, NJ], BF)
            nc.vector.tensor_tensor(out=L[:], in0=L01[:], in1=Ltl[:], op=OP.add)
            z = singles.tile([P, nt, NJ], BF)
            nc.vector.tensor_tensor(
                out=z[:], in0=L[:], in1=bc(sgn[:], nt), op=OP.mult
            )
            rl = singles.tile([P, nt, NJ], BF)
            nc.vector.tensor_scalar_max(rl[:], z[:], 0.0)
            na = singles.tile([P, nt, NJ], BF)
            nc.vector.scalar_tensor_tensor(
                out=na[:],
                in0=z[:],
                scalar=-1.0,
                in1=z[:],
                op0=OP.mult,
                op1=OP.min,
            )
            e = singles.tile([P, nt, NJ], _f32)
            nc.scalar.activation(e[:], na[:], AF.Exp)
            l = singles.tile([P, nt, NJ], _f32)
            nc.scalar.activation(l[:], e[:], AF.Ln, bias=1.0)
            sp = singles.tile([P, nt, NJ], _f32)
            nc.vector.tensor_tensor(out=sp[:], in0=rl[:], in1=l[:], op=OP.add)
            spw = singles.tile([P, nt, NJ], _f32)
            acc = singles.tile([P, 1], _f32)
            nc.vector.scalar_tensor_tensor(
                out=spw[:],
                in0=sp[:],
                scalar=1.0,
                in1=bc(wz[:], nt),
                op0=OP.mult,
                op1=OP.mult,
                accum_out=acc[:],
            )
            nc.sync.dma_start(out=out[:], in_=acc[:])

    nc.compile()
    return nc


_NC_CACHE: dict = {}


def _get_nc(nt: int):
    if nt not in _NC_CACHE:
        _NC_CACHE[nt] = build_nc(nt)
    return _NC_CACHE[nt]


def kernel(i_emb, o_emb, context, target, neg_samples, _trace=False, _trace_kwargs=None):
    i_emb = np.asarray(i_emb, dtype=np.float32)
    o_emb = np.asarray(o_emb, dtype=np.float32)
    context = np.asarray(context).astype(np.int64)
    target = np.asarray(target).astype(np.int64)
    neg_samples = np.asarray(neg_samples).astype(np.int64)

    o8 = o_emb.astype(F8NP)
    i8 = i_emb.astype(F8NP)

    allj = np.concatenate([context, neg_samples], axis=1)  # [B, 30]
    rows = o8[allj]  # [B, 30, 300]
    tg = i8[target]  # [B, 300]

    r5 = rows.reshape(NCORES, NT, P, NJ, D)
    # mv01[c, t, d, k, s, j] = r5[c, t, s, j, 128k + d]
    m1 = r5[..., : 2 * P].reshape(NCORES, NT, P, NJ, 2, P)
    mv01 = np.ascontiguousarray(m1.transpose(0, 1, 5, 4, 2, 3)).reshape(
        NCORES, NT * P, W1
    )
    # tail[c, (t, p), (j, dt)] = r5[c, t, p, j, 256 + dt]
    tail = np.ascontiguousarray(r5[..., 2 * P : D]).reshape(NCORES, NT * P, WT)

    t4 = tg.reshape(NCORES, NT, P, D)
    # st01[c, (t, d), (k, s)] = tg[c, t, s, 128k + d]
    s1 = t4[..., : 2 * P].reshape(NCORES, NT, P, 2, P)
    st01 = np.ascontiguousarray(s1.transpose(0, 1, 4, 3, 2)).reshape(
        NCORES, NT * P, 256
    )
    # tt[c, p, (t, dt)] = tg[c, t, p, 256 + dt]
    tt = np.ascontiguousarray(
        t4[..., 2 * P : D].transpose(0, 2, 1, 3)
    ).reshape(NCORES, P, NT * DT)

    # mask in (b, j) layout: 0 where b == p%16 else -inf
    pidx = np.arange(P)[:, None, None]
    bb = np.arange(16)[None, :, None]
    mrow = np.where((pidx % 16) == bb, 0.0, NEG_INF)  # [P, 16, 1]
    maskadd = np.ascontiguousarray(
        np.broadcast_to(mrow, (P, 16, NJ)).astype(BFNP)
    ).reshape(P, 16 * NJ)

    jj = np.arange(NJ)
    sgn_row = np.where(jj < C, -1.0, 1.0).astype(BFNP)
    wz_row = np.where(jj < C, 1.0 / C, 1.0).astype(np.float32)
    consts = {
        "maskadd": maskadd,
        "sgn": np.tile(sgn_row, (P, 1)),
        "wz": np.tile(wz_row, (P, 1)),
    }

    nc = _get_nc(NT)

    in_maps = []
    for c in range(NCORES):
        in_maps.append(
            {
                "mv01": mv01[c],
                "tail": tail[c],
                "st01": st01[c],
                "tt": tt[c],
                **consts,
            }
        )

    kw = {}
    if _trace:
        kw["trace"] = True
        if _trace_kwargs:
            kw.update(_trace_kwargs)
    res = run_bass_kernel_spmd(nc, in_maps, core_ids=list(range(NCORES)), **kw)

    total = np.float64(0.0)
    for c in range(NCORES):
        total += np.asarray(res.results[c]["out"], dtype=np.float64).sum()
    loss = np.float32(total / B)
    if _trace:
        return loss, res
    return loss


# revision 8
# speedup vs baseline: 2.2286x; 1.1109x over previous
"""CBOW negative-sampling loss on 8 Trainium2 NeuronCores — v6.

Measured constraints driving this design (HW microbenchmarks):
  - per-core DMA ceiling ~330 GB/s (two HWDGE rings help <15%): fp8
    packing is mandatory (19MB/core -> ~58us floor; bf16 would be 117).
  - DVE is 2 elem/lane/cyc only for 2-byte dtypes (fp8 runs 1x).
  - DoubleRow fp8 matmul: 480-col MM sustains ~365ns back-to-back
    (200ns stream + ~165ns fixed); LDWEIGHTS hides behind the previous
    MM. DR requires PSUM dst partition offset 0.

Pipeline per 128-sample tile (4 groups of 32 samples):
  - PE (8 DR matmuls): d 0:256 dot-product parts. Group q's stationary
    is full-width [128,2,128] with the 32 target vectors at columns
    32q:32q+32, zeros elsewhere (memset-once A/B buffers; per-tile DMA
    rewrites only the data blocks). Groups accumulate into one
    [128, 2x480] PSUM region; cell [m, h, b*30+j] holds
    tgt_m . row_{32(m//32)+16h+b}[0:256] — diagonal m%32==16h+b is the
    wanted partial logit.
  - ACT: evacuate PSUM -> bf16 Y reordered to [j, (h,b)] layout (free
    strided reorder), and upcast the fp8 d 256:300 tail rows to bf16.
  - DVE (all 2x mode): Y + maskconst (0 on diagonal, -1e30 off) then
    tensor_reduce(max) over (h,b) -> L01[:, t, 30]. Tail: bf16
    multiply by broadcast target tail + reduce(add) over d44 ->
    Ltail[:, t, 30].
  - Post: L = L01 + Ltail, sign flip, stable softplus, weighted
    accumulate -> [128,1]; host sums across cores /B.
"""

import sys

for _p in ("/opt/trn_rl_repo", "/opt/pypackages"):
    if _p not in sys.path:
        sys.path.append(_p)

import ml_dtypes
import numpy as np

import concourse.bass as bass
import concourse.bacc as bacc
import concourse.tile as tile
from concourse import mybir
from concourse.bass_utils import run_bass_kernel_spmd

V = 100000
D = 300
B = 16384
C = 10
K = 20
NCORES = 8
P = 128
NJ = C + K  # 30
BCORE = B // NCORES  # 2048
NT = BCORE // P  # 16
DT = 44  # tail dims 256:300
W1 = 2 * NJ * P  # 7680
WT = NJ * DT  # 1320
NEG_INF = -1.0e30

F8NP = ml_dtypes.float8_e4m3
BFNP = ml_dtypes.bfloat16
F8 = mybir.dt.float8e4
BF = mybir.dt.bfloat16
_f32 = mybir.dt.float32
DR = mybir.MatmulPerfMode.DoubleRow


def _ap(sliced, dims):
    return bass.AP(sliced.tensor, sliced.offset, [sliced.ap[0], *dims])


def build_nc(nt: int):
    nc = bacc.Bacc(None, target_bir_lowering=False, debug=False)
    AF = mybir.ActivationFunctionType
    OP = mybir.AluOpType

    mv01 = nc.dram_tensor("mv01", [nt * P, W1], F8, kind="ExternalInput")
    taild = nc.dram_tensor("tail", [nt * P, WT], F8, kind="ExternalInput")
    st01d2 = nc.dram_tensor("st01", [P, nt * 256], F8, kind="ExternalInput")
    ttd = nc.dram_tensor("tt", [P, nt * DT], F8, kind="ExternalInput")
    maskd = nc.dram_tensor("maskadd", [P, 16 * NJ], BF, kind="ExternalInput")
    sgnd = nc.dram_tensor("sgn", [P, NJ], BF, kind="ExternalInput")
    wzd = nc.dram_tensor("wz", [P, NJ], _f32, kind="ExternalInput")
    out = nc.dram_tensor("out", [P, 1], _f32, kind="ExternalOutput")

    with tile.TileContext(nc) as tc:
        with (
            tc.tile_pool(name="g1p", bufs=4) as g1p,
            tc.tile_pool(name="tlp", bufs=4) as tlp,
            tc.tile_pool(name="tbp", bufs=2) as tbp,
            tc.tile_pool(name="tpp", bufs=2) as tpp,
            tc.tile_pool(name="yp", bufs=2) as yp,
            tc.tile_pool(name="mp", bufs=2) as mp,
            tc.tile_pool(name="pp", bufs=2, space="PSUM") as pp,
            tc.tile_pool(name="singles", bufs=1) as singles,
        ):
            mask = singles.tile([P, 16 * NJ], BF)
            nc.scalar.dma_start(out=mask[:], in_=maskd[:])
            sgn = singles.tile([P, NJ], BF)
            nc.scalar.dma_start(out=sgn[:], in_=sgnd[:])
            wz = singles.tile([P, NJ], _f32)
            nc.scalar.dma_start(out=wz[:], in_=wzd[:])
            tt8 = singles.tile([P, nt, DT], F8)
            nc.scalar.dma_start(out=tt8[:], in_=ttd[:])
            st01s = singles.tile([P, nt, 2, P], F8)
            nc.scalar.dma_start(out=st01s[:], in_=st01d2[:])
            ttb = singles.tile([P, nt, DT], BF)
            nc.scalar.activation(ttb[:], tt8[:], AF.Copy)

            # A/B zero-padded stationaries: [d, k, q, 128], data blocks
            # at plane-q cols 32q:32q+32 (flat (q,128)-space stride 160)
            stb = []
            for i in range(2):
                s_ = singles.tile([P, 2, 8, P], F8, tag=f"stb{i}")
                nc.vector.memset(s_[:], 0.0)
                stb.append(s_)

            L01 = singles.tile([P, nt, NJ], BF)
            Ltl = singles.tile([P, nt, NJ], BF)

            for t in range(nt):
                g1 = g1p.tile([P, 2, NJ * P], F8, tag="g1")
                nc.sync.dma_start(
                    out=g1[:, 0, :], in_=mv01[t * P : (t + 1) * P, 0 : NJ * P]
                )
                nc.scalar.dma_start(
                    out=g1[:, 1, :],
                    in_=mv01[t * P : (t + 1) * P, NJ * P : W1],
                )
                tl8 = tlp.tile([P, NJ, DT], F8, tag="tl8")
                nc.sync.dma_start(out=tl8[:], in_=taild[t * P : (t + 1) * P])

                # ACT scatter-copy of this tile's targets into the
                # zero-padded stationary ((k, q, b) -> strided blocks)
                st = stb[t % 2]
                nc.scalar.activation(
                    _ap(st[:, 0, 0, 0:16], [[1024, 2], [144, 8], [1, 16]]),
                    _ap(st01s[:, t, 0, 0:16], [[P, 2], [16, 8], [1, 16]]),
                    AF.Copy,
                )

                ps = pp.tile([P, 512], _f32, tag="ps")
                o = ps[:, 0 : 16 * NJ]
                for q in range(8):
                    c0 = 480 * q
                    nc.tensor.matmul(
                        o,
                        st[:, :, q, :],
                        g1[:, :, c0 : c0 + 480],
                        start=(q == 0),
                        stop=(q == 7),
                        perf_mode=DR,
                    )

                # ACT: contiguous evac psum -> Y bf16 [P, (b, j)]
                Y = yp.tile([P, 16 * NJ], BF, tag="Y")
                nc.scalar.activation(Y[:], ps[:, 0 : 16 * NJ], AF.Copy)
                # ACT: upcast tail rows fp8 -> bf16
                tlb = tbp.tile([P, NJ, DT], BF, tag="tlb")
                nc.scalar.activation(tlb[:], tl8[:], AF.Copy)

                # DVE: masked max-extraction of the diagonal (b == p%16)
                M = mp.tile([P, 16 * NJ], BF, tag="M")
                nc.vector.tensor_tensor(
                    out=M[:], in0=Y[:], in1=mask[:], op=OP.add
                )
                nc.vector.tensor_reduce(
                    out=L01[:, t, :],
                    in_=_ap(M[:], [[1, NJ], [NJ, 16]]),
                    axis=mybir.AxisListType.X,
                    op=OP.max,
                )

                # GPSIMD: tail products; DVE: reduce over d44
                tp = tpp.tile([P, NJ, DT], BF, tag="tp")
                nc.gpsimd.tensor_tensor(
                    out=tp[:],
                    in0=tlb[:],
                    in1=_ap(ttb[:, t, :], [[0, NJ], [1, DT]]),
                    op=OP.mult,
                )
                tf = tpp.tile([P, NJ, DT // 2], BF, tag="tf")
                nc.vector.tensor_tensor(
                    out=tf[:],
                    in0=tp[:, :, 0 : DT // 2],
                    in1=tp[:, :, DT // 2 : DT],
                    op=OP.add,
                )
                with nc.allow_low_precision("tail sums fit bf16"):
                    nc.vector.tensor_reduce(
                        out=Ltl[:, t, :],
                        in_=tf[:],
                        axis=mybir.AxisListType.X,
                        op=OP.add,
                    )

            # ---- post-pass: [P, nt*30] ----
            def bc(a, n):
                return bass.AP(a.tensor, a.offset, [a.ap[0], [0, n], a.ap[-1]])

            L = singles.tile([P, nt, NJ], BF)
            nc.vector.tensor_tensor(out=L[:], in0=L01[:], in1=Ltl[:], op=OP.add)
            z = singles.tile([P, nt, NJ], BF)
            nc.vector.tensor_tensor(
                out=z[:], in0=L[:], in1=bc(sgn[:], nt), op=OP.mult
            )
            rl = singles.tile([P, nt, NJ], BF)
            nc.vector.tensor_scalar_max(rl[:], z[:], 0.0)
            na = singles.tile([P, nt, NJ], BF)
            nc.vector.scalar_tensor_tensor(
                out=na[:],
                in0=z[:],
                scalar=-1.0,
                in1=z[:],
                op0=OP.mult,
                op1=OP.min,
            )
            e = singles.tile([P, nt, NJ], _f32)
            nc.scalar.activation(e[:], na[:], AF.Exp)
            l = singles.tile([P, nt, NJ], _f32)
            nc.scalar.activation(l[:], e[:], AF.Ln, bias=1.0)
            sp = singles.tile([P, nt, NJ], _f32)
            nc.vector.tensor_tensor(out=sp[:], in0=rl[:], in1=l[:], op=OP.add)
            spw = singles.tile([P, nt, NJ], _f32)
            acc = singles.tile([P, 1], _f32)
            nc.vector.scalar_tensor_tensor(
                out=spw[:],
                in0=sp[:],
                scalar=1.0,
                in1=bc(wz[:], nt),
                op0=OP.mult,
                op1=OP.mult,
                accum_out=acc[:],
            )
            nc.sync.dma_start(out=out[:], in_=acc[:])

    nc.compile()
    return nc


_NC_CACHE: dict = {}


def _get_nc(nt: int):
    if nt not in _NC_CACHE:
        _NC_CACHE[nt] = build_nc(nt)
    return _NC_CACHE[nt]


def kernel(i_emb, o_emb, context, target, neg_samples, _trace=False, _trace_kwargs=None):
    i_emb = np.asarray(i_emb, dtype=np.float32)
    o_emb = np.asarray(o_emb, dtype=np.float32)
    context = np.asarray(context).astype(np.int64)
    target = np.asarray(target).astype(np.int64)
    neg_samples = np.asarray(neg_samples).astype(np.int64)

    o8 = o_emb.astype(F8NP)
    i8 = i_emb.astype(F8NP)

    allj = np.concatenate([context, neg_samples], axis=1)  # [B, 30]
    rows = o8[allj]  # [B, 30, 300]
    tg = i8[target]  # [B, 300]

    r5 = rows.reshape(NCORES, NT, P, NJ, D)
    # mv01[c, t, d, k, s, j] = r5[c, t, s, j, 128k + d]
    m1 = r5[..., : 2 * P].reshape(NCORES, NT, P, NJ, 2, P)
    mv01 = np.ascontiguousarray(m1.transpose(0, 1, 5, 4, 2, 3)).reshape(
        NCORES, NT * P, W1
    )
    # tail[c, (t, p), (j, dt)] = r5[c, t, p, j, 256 + dt]
    tail = np.ascontiguousarray(r5[..., 2 * P : D]).reshape(NCORES, NT * P, WT)

    t4 = tg.reshape(NCORES, NT, P, D)
    # st01[c, d, (t, k, s)] = tg[c, t, s, 128k + d]
    s1 = t4[..., : 2 * P].reshape(NCORES, NT, P, 2, P)
    st01 = np.ascontiguousarray(s1.transpose(0, 4, 1, 3, 2)).reshape(
        NCORES, P, NT * 256
    )
    # tt[c, p, (t, dt)] = tg[c, t, p, 256 + dt]
    tt = np.ascontiguousarray(
        t4[..., 2 * P : D].transpose(0, 2, 1, 3)
    ).reshape(NCORES, P, NT * DT)

    # mask in (b, j) layout: 0 where b == p%16 else -inf
    pidx = np.arange(P)[:, None, None]
    bb = np.arange(16)[None, :, None]
    mrow = np.where((pidx % 16) == bb, 0.0, NEG_INF)  # [P, 16, 1]
    maskadd = np.ascontiguousarray(
        np.broadcast_to(mrow, (P, 16, NJ)).astype(BFNP)
    ).reshape(P, 16 * NJ)

    jj = np.arange(NJ)
    sgn_row = np.where(jj < C, -1.0, 1.0).astype(BFNP)
    wz_row = np.where(jj < C, 1.0 / C, 1.0).astype(np.float32)
    consts = {
        "maskadd": maskadd,
        "sgn": np.tile(sgn_row, (P, 1)),
        "wz": np.tile(wz_row, (P, 1)),
    }

    nc = _get_nc(NT)

    in_maps = []
    for c in range(NCORES):
        in_maps.append(
            {
                "mv01": mv01[c],
                "tail": tail[c],
                "st01": st01[c],
                "tt": tt[c],
                **consts,
            }
        )

    kw = {}
    if _trace:
        kw["trace"] = True
        if _trace_kwargs:
            kw.update(_trace_kwargs)
    res = run_bass_kernel_spmd(nc, in_maps, core_ids=list(range(NCORES)), **kw)

    total = np.float64(0.0)
    for c in range(NCORES):
        total += np.asarray(res.results[c]["out"], dtype=np.float64).sum()
    loss = np.float32(total / B)
    if _trace:
        return loss, res
    return loss
